# revision 1
# baseline (speedup 1.0000x reference)
"""Trainium2 Bass kernel for nn_MultiHeadAttention (B=2, S=2048, D=1024, H=16, dk=64).

Sharding: 8 cores = (batch b in {0,1}) x (head group g in {0..3}, 4 heads each).
Key observation: the reference does a RAW reshape (B,H,S,dk) -> (B,S,H*dk)
(mixing head and sequence axes), so output row s' = h*128 + s//16 of X @ WO
depends ONLY on head h.  Core (b,g) therefore produces output rows
[512g, 512(g+1)) of batch b -- a pure concatenation, no collectives.

Per-core pipeline (all matmuls float32r, fp32 PSUM accumulate):
  1. QpT/KpT = W^T @ x^T   -> [heads*dk, S] layouts (host pre-transposes Q,K,V)
     Vp      = x^T.T @ Wv  -> [S, heads*dk] natural layout, augmented with a
               ones column per head (denominator trick).
  2. Per head pair, per s_q block of 512:
     scores^T[s_k, s_q] tiles on PE (2 heads packed via row tile_position),
     exp via ACT (scale=1/8 fused, no max subtraction -- fp32 exp is safe
     for |score*scale| <~ 25), P@V with V_aug -> PSUM [65, 512] where row 64
     is the softmax denominator.
  3. recip (DVE) -> broadcast via K=1 outer-product matmul -> normalize on DVE,
     writing the head/seq-mixed layout directly with a strided AP.
  4. One scatter DMA per (head, partition parity) assembles X^T tiles;
     WO matmul per head; DMA out.
"""

import sys

try:
    import concourse.bass as bass  # noqa: F401
except ImportError:
    sys.path.insert(0, "/opt/trn_rl_repo")

import numpy as np

import concourse.bacc as bacc
import concourse.tile as tile
from concourse import mybir
from concourse.bass_utils import run_bass_kernel_spmd

F32R = mybir.dt.float32r
F32 = mybir.dt.float32

B, S, D, H, DK = 2, 2048, 1024, 16, 64
HEADS_PER_CORE = 4
GROUPS = 4
SCALE = 1.0 / 8.0  # 1/sqrt(dk)
E_BUFS = 12

_cached_nc = None


def build_nc():
    nc = bacc.Bacc(None, target_bir_lowering=False)
    qT = nc.dram_tensor("qT", [D, S], F32R, kind="ExternalInput")
    kT = nc.dram_tensor("kT", [D, S], F32R, kind="ExternalInput")
    vT = nc.dram_tensor("vT", [D, S], F32R, kind="ExternalInput")
    wq = nc.dram_tensor("wq", [D, 256], F32R, kind="ExternalInput")
    wk = nc.dram_tensor("wk", [D, 256], F32R, kind="ExternalInput")
    wv = nc.dram_tensor("wv", [D, 256], F32R, kind="ExternalInput")
    wo = nc.dram_tensor("wo", [D, D], F32R, kind="ExternalInput")
    out = nc.dram_tensor("out", [512, D], F32, kind="ExternalOutput")

    Exp = mybir.ActivationFunctionType.Exp
    BF16 = mybir.dt.bfloat16

    with tile.TileContext(nc) as tc, nc.allow_low_precision(
        reason="float32r tiles hold full fp32 bits; attention weights in bf16 "
        "average out over 2048 positions; PSUM accumulation is fp32"
    ):
        with (
            tc.tile_pool(name="persist", bufs=1) as persist,
            tc.tile_pool(name="hrp", bufs=2) as hrp,
            tc.tile_pool(name="xhp", bufs=2) as xhp,
            tc.tile_pool(name="small", bufs=2) as small,
            tc.tile_pool(name="opool", bufs=2) as opool,
            tc.tile_pool(name="epool", bufs=E_BUFS) as epool,
            tc.tile_pool(name="ps_mix", bufs=2, space="PSUM") as ps_mix,
            tc.tile_pool(name="ps_sc", bufs=2, space="PSUM") as ps_sc,
            tc.tile_pool(name="ps_pv", bufs=2, space="PSUM") as ps_pv,
        ):
            qpT = persist.tile([128, 2, S], F32R, tag="qpT")
            kpT = persist.tile([128, 2, S], F32R, tag="kpT")
            vaug = persist.tile([128, 16, 4, 65], F32R, tag="vaug")
            ones_f32 = persist.tile([128, 1], F32, tag="ones_f32")
            nc.vector.memset(ones_f32, 1.0)
            nc.vector.tensor_copy(
                vaug[:, :, :, 64:65], ones_f32.to_broadcast((128, 16, 4, 1))
            )
            ones = persist.tile([1, 64], F32R, tag="ones")
            nc.vector.tensor_copy(ones, ones_f32[0:1, :].to_broadcast((1, 64)))

            # ---------------- Phase A: projections ----------------
            # Emission order sets scheduler priority: K first (scores lhsT
            # needs all of it), then Q block 0 (first scores rhs), then V
            # (P@V), then remaining Q blocks, then WO weights.
            with (
                tc.tile_pool(name="wqkv", bufs=1) as wqkv,
                tc.tile_pool(name="stream", bufs=3) as stream,
            ):
                wq_sb = wqkv.tile([128, 8, 256], F32R, tag="wq")
                wk_sb = wqkv.tile([128, 8, 256], F32R, tag="wk")
                wv_sb = wqkv.tile([128, 8, 256], F32R, tag="wv")
                for w_dram, w_sb in ((wk, wk_sb), (wq, wq_sb), (wv, wv_sb)):
                    nc.sync.dma_start(
                        out=w_sb, in_=w_dram.rearrange("(t p) n -> p t n", p=128)
                    )

                def proj_qk(x_dram, w_sb, outt, nb):
                    st = stream.tile([128, 8, 512], F32R, tag="acts", name="stq")
                    nc.sync.dma_start(
                        out=st,
                        in_=x_dram.rearrange("(t p) s -> p t s", p=128)[
                            :, :, 512 * nb : 512 * (nb + 1)
                        ],
                    )
                    for m in range(2):
                        ps = ps_mix.tile([128, 512], F32, tag="mix", name="psq")
                        for k in range(8):
                            nc.tensor.matmul(
                                ps,
                                w_sb[:, k, 128 * m : 128 * (m + 1)],
                                st[:, k, :],
                                start=(k == 0),
                                stop=(k == 7),
                            )
                        nc.vector.tensor_copy(
                            outt[:, m, 512 * nb : 512 * (nb + 1)], ps
                        )

                def proj_v(nb):
                    st = stream.tile([128, 8, 512], F32R, tag="acts", name="stv")
                    nc.sync.dma_start(
                        out=st,
                        in_=vT.rearrange("(t p) s -> p t s", p=128)[
                            :, :, 512 * nb : 512 * (nb + 1)
                        ],
                    )
                    for sti in range(4):
                        stt = 4 * nb + sti
                        ps_full = ps_mix.tile([128, 512], F32, tag="mix", name="vps")
                        ps = ps_full[:, :256]
                        for k in range(8):
                            nc.tensor.matmul(
                                ps,
                                st[:, k, 128 * sti : 128 * (sti + 1)],
                                wv_sb[:, k, :],
                                start=(k == 0),
                                stop=(k == 7),
                            )
                        nc.vector.tensor_copy(
                            vaug[:, stt, :, 0:64],
                            ps.rearrange("p (h c) -> p h c", h=4),
                        )

                for nb in range(4):
                    proj_qk(kT, wk_sb, kpT, nb)
                proj_qk(qT, wq_sb, qpT, 0)
                for nb in range(4):
                    proj_v(nb)
                for nb in range(1, 4):
                    proj_qk(qT, wq_sb, qpT, nb)

            # ------------- Phase B/C: attention + output projection -------------
            with tc.tile_pool(name="wop", bufs=1) as wop:
                wo_sb = wop.tile([128, 8, D], F32R, tag="wo")
                nc.sync.dma_start(
                    out=wo_sb, in_=wo.rearrange("(t p) n -> p t n", p=128)
                )

                for hp in range(2):
                    hA, hB = 2 * hp, 2 * hp + 1
                    hr = {
                        h: hrp.tile([64, 2048], F32R, tag="hr", name=f"hr{h}")
                        for h in (hA, hB)
                    }
                    for qb in range(4):
                        pv = {
                            h: ps_pv.tile([65, 512], F32, tag="pv", name=f"pv{h}")
                            for h in (hA, hB)
                        }
                        for kp in range(8):  # pairs of k tiles
                            sc = {
                                h: ps_sc.tile([128, 1024], F32, tag="sc",
                                              name=f"sc{h}")
                                for h in (hA, hB)
                            }
                            for half in range(2):
                                kt = 2 * kp + half
                                for i, h in enumerate((hA, hB)):
                                    nc.tensor.matmul(
                                        sc[h][:, 512 * half : 512 * (half + 1)],
                                        kpT[64 * i : 64 * (i + 1), hp,
                                            128 * kt : 128 * (kt + 1)],
                                        qpT[64 * i : 64 * (i + 1), hp,
                                            512 * qb : 512 * (qb + 1)],
                                        start=True,
                                        stop=True,
                                        tile_position=(64 * i, 0),
                                    )
                            e = {}
                            for h in (hA, hB):
                                e[h] = epool.tile([128, 1024], F32R, tag="e",
                                                  name=f"e{h}")
                                nc.scalar.activation(e[h], sc[h], Exp, scale=SCALE)
                            for half in range(2):
                                kt = 2 * kp + half
                                for h in (hA, hB):
                                    nc.tensor.matmul(
                                        pv[h],
                                        vaug[:, kt, h, :],
                                        e[h][:, 512 * half : 512 * (half + 1)],
                                        start=(kt == 0),
                                        stop=(kt == 15),
                                    )
                        # normalize + scatter-layout write
                        for h in (hA, hB):
                            rc = small.tile([1, 512], F32R, tag="rc", name=f"rc{h}")
                            nc.vector.reciprocal(rc, pv[h][64:65, :])
                            bct = ps_sc.tile([128, 1024], F32, tag="sc",
                                             name=f"bct{h}")
                            bc = bct[0:64, 0:512]
                            nc.tensor.matmul(bc, ones, rc, start=True, stop=True)
                            bc_sb = small.tile([64, 512], F32, tag="bcs",
                                               name=f"bcs{h}")
                            nc.vector.tensor_copy(bc_sb, bc)
                            hview = hr[h].rearrange("p (j r) -> p r j", j=16)[
                                :, 32 * qb : 32 * (qb + 1), :
                            ]
                            nc.vector.tensor_mul(hview, pv[h][0:64, :], bc_sb)

                    # scatter DMA into X^T layout + WO matmul per head
                    for h in (hA, hB):
                        xh = xhp.tile([128, 8, 128], F32R, tag="xh", name=f"xh{h}")
                        hv = hr[h].rearrange("p (j r) -> p j r", j=16)
                        for par in range(2):
                            nc.sync.dma_start(
                                out=xh[64 * par : 64 * (par + 1)],
                                in_=hv[:, par::2, :],
                            )
                        for n in range(2):
                            wops = ps_mix.tile([128, 512], F32, tag="mix",
                                               name=f"wops{h}")
                            for t in range(8):
                                nc.tensor.matmul(
                                    wops,
                                    xh[:, t, :],
                                    wo_sb[:, t, 512 * n : 512 * (n + 1)],
                                    start=(t == 0),
                                    stop=(t == 7),
                                )
                            ot = opool.tile([128, 512], F32, tag="o", name=f"ot{h}")
                            nc.vector.tensor_copy(ot, wops)
                            nc.sync.dma_start(
                                out=out[128 * h : 128 * (h + 1),
                                        512 * n : 512 * (n + 1)],
                                in_=ot,
                            )

    nc.finalize()
    return nc


def make_in_maps(Q, K, V, WQ, WK, WV, WO):
    in_maps = []
    wo_full = np.ascontiguousarray(WO.astype(np.float32))
    for b in range(B):
        qTb = np.ascontiguousarray(Q[b].T.astype(np.float32))
        kTb = np.ascontiguousarray(K[b].T.astype(np.float32))
        vTb = np.ascontiguousarray(V[b].T.astype(np.float32))
        for g in range(GROUPS):
            hs = slice(4 * g, 4 * g + 4)
            # [4, D, dk] -> [D, 4*dk]
            wqc = np.ascontiguousarray(
                WQ[hs].transpose(1, 0, 2).reshape(D, 256).astype(np.float32)
            )
            wkc = np.ascontiguousarray(
                WK[hs].transpose(1, 0, 2).reshape(D, 256).astype(np.float32)
            )
            wvc = np.ascontiguousarray(
                WV[hs].transpose(1, 0, 2).reshape(D, 256).astype(np.float32)
            )
            in_maps.append(
                {"qT": qTb, "kT": kTb, "vT": vTb,
                 "wq": wqc, "wk": wkc, "wv": wvc, "wo": wo_full}
            )
    return in_maps


def run(inputs, **run_kwargs):
    global _cached_nc
    if _cached_nc is None:
        _cached_nc = build_nc()
    in_maps = make_in_maps(**inputs)
    res = run_bass_kernel_spmd(
        _cached_nc, in_maps, core_ids=list(range(8)), **run_kwargs
    )
    full = np.zeros((B, S, D), np.float32)
    for b in range(B):
        for g in range(GROUPS):
            full[b, 512 * g : 512 * (g + 1), :] = res.results[4 * b + g]["out"]
    return full, res


def kernel(**inputs):
    full, _ = run(inputs)
    return full


if __name__ == "__main__":
    rng = np.random.default_rng(0)
    inputs = {
        "Q": rng.standard_normal((B, S, D)).astype(np.float32),
        "K": rng.standard_normal((B, S, D)).astype(np.float32),
        "V": rng.standard_normal((B, S, D)).astype(np.float32),
        "WQ": (rng.uniform(-0.1, 0.1, (H, D, DK))).astype(np.float32),
        "WK": (rng.uniform(-0.1, 0.1, (H, D, DK))).astype(np.float32),
        "WV": (rng.uniform(-0.1, 0.1, (H, D, DK))).astype(np.float32),
        "WO": (rng.uniform(-0.1, 0.1, (H * DK, D))).astype(np.float32),
    }
    out = kernel(**inputs)
    print("kernel out", out.shape, out.dtype, float(np.abs(out).max()))



# revision 11
# speedup vs baseline: 1.5401x; 1.5401x over previous
"""Trainium2 Bass kernel for nn_MultiHeadAttention (B=2, S=2048, D=1024, H=16, dk=64).

Sharding: 8 cores = (batch b in {0,1}) x (head group g in {0..3}, 4 heads each).
The reference's RAW reshape (B,H,S,dk) -> (B,S,H*dk) means output row
s' = h*128 + s//16 depends only on head h, so core (b,g) produces output rows
[512g, 512(g+1)) of batch b -- pure concatenation, no collectives.

v1 (bf16): all matmul operands bf16 (fp32 runs the PE in 4-pass HIGH mode;
bf16 streams 1 column/cycle), softmax denominators via the augmented-ones
column of V, reciprocal via the fast-approx DVE op, and the whole kernel is
software-pipelined: projections and output matmuls are interleaved as filler
into the ACT-paced attention loop so the tensor engine never idles.

Per-core pipeline:
  1. QpT/KpT = W^T @ x^T  -> [128(=2 heads*dk), 2(head pair), S] bf16
     Vaug    = x^T.T @ Wv -> [128 kpos-chunk, 16 kt, 4 heads, 64+1] bf16
     (ones column -> softmax denominator accumulates in the P@V matmul)
  2. Per head pair, per 512-wide q block: scores^T tiles on PE (2 heads
     packed via row tile_position), exp via ACT (scale fused, no max
     subtraction -- fp32 PSUM scores, bf16 exp output), P@V with V_aug
     -> PSUM [65, 512], row 64 = denominator.
  3. recip_approx_fast (DVE) -> bf16 -> broadcast via K=1 outer-product
     matmul -> normalize on DVE, writing head/seq-mixed layout directly.
  4. One scatter DMA per (head, partition parity) assembles X^T tiles;
     WO matmul per head; DMA out fp32.
"""

import sys

try:
    import concourse.bass as bass  # noqa: F401
except ImportError:
    sys.path.insert(0, "/opt/trn_rl_repo")

import os

import numpy as np
from ml_dtypes import bfloat16

FILLER = os.environ.get("K_FILLER", "1") == "1"
RECIP_EXACT = os.environ.get("K_RECIP_EXACT", "1") == "1"

import concourse.bacc as bacc
import concourse.tile as tile
from concourse import mybir
from concourse.bass_utils import run_bass_kernel_spmd

BF = mybir.dt.bfloat16
F16 = mybir.dt.float16
F32 = mybir.dt.float32

B, S, D, H, DK = 2, 2048, 1024, 16, 64
GROUPS = 4
SCALE = 1.0 / 8.0  # 1/sqrt(dk)
E_BUFS = 12

_cached_nc = None


def build_nc():
    nc = bacc.Bacc(None, target_bir_lowering=False)
    qT = nc.dram_tensor("qT", [D, S], F16, kind="ExternalInput")
    kT = nc.dram_tensor("kT", [D, S], F16, kind="ExternalInput")
    vT = nc.dram_tensor("vT", [D, S], F16, kind="ExternalInput")
    wq = nc.dram_tensor("wq", [D, 256], F16, kind="ExternalInput")
    wk = nc.dram_tensor("wk", [D, 256], F16, kind="ExternalInput")
    wv = nc.dram_tensor("wv", [D, 256], F16, kind="ExternalInput")
    wo = nc.dram_tensor("wo", [D, D], F16, kind="ExternalInput")
    out = nc.dram_tensor("out", [512, D], F32, kind="ExternalOutput")

    Exp = mybir.ActivationFunctionType.Exp

    with tile.TileContext(nc) as tc, nc.allow_low_precision(
        reason="bf16 matmuls with fp32 PSUM accumulation; attention weight "
        "and normalization rounding averages out over 2048 positions"
    ):
        with (
            tc.tile_pool(name="persist", bufs=1) as persist,
            tc.tile_pool(name="wqkv", bufs=1) as wqkv,
            tc.tile_pool(name="hrp", bufs=4) as hrp,
            tc.tile_pool(name="xhp", bufs=2) as xhp,
            tc.tile_pool(name="small", bufs=4) as small,
            tc.tile_pool(name="opool", bufs=2) as opool,
            tc.tile_pool(name="epool", bufs=E_BUFS) as epool,
            tc.tile_pool(name="stream", bufs=3) as stream,
            tc.tile_pool(name="vstream", bufs=2) as vstream,
            tc.tile_pool(name="ps_mix", bufs=2, space="PSUM") as ps_mix,
            tc.tile_pool(name="ps_sc", bufs=2, space="PSUM") as ps_sc,
            tc.tile_pool(name="ps_pv", bufs=2, space="PSUM") as ps_pv,
        ):
            qpT = persist.tile([128, 2, S], F16, tag="qpT")
            kpT = persist.tile([128, 2, S], F16, tag="kpT")
            vaug = persist.tile([128, 16, 4, 65], BF, tag="vaug")
            ones_f32 = persist.tile([128, 1], F32, tag="ones_f32")
            nc.vector.memset(ones_f32, 1.0)
            nc.vector.tensor_copy(
                vaug[:, :, :, 64:65], ones_f32.to_broadcast((128, 16, 4, 1))
            )
            ones_bf = persist.tile([1, 64], BF, tag="ones_bf")
            nc.vector.tensor_copy(ones_bf, ones_f32[0:1, :].to_broadcast((1, 64)))

            wq_sb = wqkv.tile([128, 8, 256], F16, tag="wq")
            wk_sb = wqkv.tile([128, 8, 256], F16, tag="wk")
            wv_sb = wqkv.tile([128, 8, 256], F16, tag="wv")
            wo_sb = wqkv.tile([128, 8, D], F16, tag="wo")
            for w_dram, w_sb in ((wk, wk_sb), (wq, wq_sb), (wv, wv_sb)):
                nc.sync.dma_start(
                    out=w_sb, in_=w_dram.rearrange("(t p) n -> p t n", p=128)
                )

            # --- emission helpers; each is one "filler group" (~1us PE) ---

            def emit_qk_proj(x_dram, w_sb, outt, m, nb):
                """One [128, 512] block of a Q/K projection for head pair m."""
                st = stream.tile([128, 8, 512], F16, tag="acts", name="stq")
                nc.sync.dma_start(
                    out=st,
                    in_=x_dram.rearrange("(t p) s -> p t s", p=128)[
                        :, :, 512 * nb : 512 * (nb + 1)
                    ],
                )
                ps = ps_mix.tile([128, 512], F32, tag="mix", name="psq")
                for k in range(8):
                    nc.tensor.matmul(
                        ps,
                        w_sb[:, k, 128 * m : 128 * (m + 1)],
                        st[:, k, :],
                        start=(k == 0),
                        stop=(k == 7),
                    )
                nc.vector.tensor_copy(outt[:, m, 512 * nb : 512 * (nb + 1)], ps)

            v_st = {}

            def emit_v_group(kt):
                """V projection for one 128-row kpos chunk kt (all 4 heads)."""
                nb = kt // 4
                if nb not in v_st:
                    st = vstream.tile([128, 8, 512], F16, tag="vacts", name="stv")
                    nc.sync.dma_start(
                        out=st,
                        in_=vT.rearrange("(t p) s -> p t s", p=128)[
                            :, :, 512 * nb : 512 * (nb + 1)
                        ],
                    )
                    v_st[nb] = st
                st = v_st[nb]
                sti = kt % 4
                ps_full = ps_mix.tile([128, 512], F32, tag="mix", name="vps")
                ps = ps_full[:, :256]
                for k in range(8):
                    nc.tensor.matmul(
                        ps,
                        st[:, k, 128 * sti : 128 * (sti + 1)],
                        wv_sb[:, k, :],
                        start=(k == 0),
                        stop=(k == 7),
                    )
                nc.vector.tensor_copy(
                    vaug[:, kt, :, 0:64], ps.rearrange("p (h c) -> p h c", h=4)
                )

            def emit_wo(h, xh):
                """Output projection for head h from assembled X^T tiles."""
                for n in range(2):
                    wops = ps_mix.tile([128, 512], F32, tag="mix", name=f"wops{h}")
                    for t in range(8):
                        nc.tensor.matmul(
                            wops,
                            xh[:, t, :],
                            wo_sb[:, t, 512 * n : 512 * (n + 1)],
                            start=(t == 0),
                            stop=(t == 7),
                        )
                    ot = opool.tile([128, 512], F32, tag="o", name=f"ot{h}")
                    nc.vector.tensor_copy(ot, wops)
                    nc.sync.dma_start(
                        out=out[128 * h : 128 * (h + 1), 512 * n : 512 * (n + 1)],
                        in_=ot,
                    )

            def emit_scatter(h, hr_h):
                """hr (head-transposed) -> X^T chunks for the WO lhsT."""
                xh = xhp.tile([128, 8, 128], F16, tag="xh", name=f"xh{h}")
                hv = hr_h.rearrange("p (j r) -> p j r", j=16)
                for par in range(2):
                    nc.sync.dma_start(
                        out=xh[64 * par : 64 * (par + 1)], in_=hv[:, par::2, :]
                    )
                return xh

            # ---------------- software-pipelined main schedule ----------------
            # Pre-phase: K proj (pair 0), Q proj (pair 0, block 0), V kt 0-1.
            for nb in range(4):
                emit_qk_proj(kT, wk_sb, kpT, 0, nb)
            emit_qk_proj(qT, wq_sb, qpT, 0, 0)
            emit_v_group(0)
            emit_v_group(1)
            if not FILLER:
                for kt in range(2, 16):
                    emit_v_group(kt)
                for nb in range(1, 4):
                    emit_qk_proj(qT, wq_sb, qpT, 0, nb)
                for nb in range(4):
                    emit_qk_proj(kT, wk_sb, kpT, 1, nb)
                for nb in range(4):
                    emit_qk_proj(qT, wq_sb, qpT, 1, nb)
            # wo needed ~150us in; emitted here so it doesn't delay kT/qT
            nc.sync.dma_start(
                out=wo_sb, in_=wo.rearrange("(t p) n -> p t n", p=128)
            )

            # Filler queues per (hp, qb) attention block: list of closures.
            # RAW constraint: a filler producing qpT/kpT block X must be
            # emitted in a block STRICTLY BEFORE the attention block whose
            # scores read X (engines run in-order; qb's sc matmuls are
            # emitted before qb's fillers).
            filler = {
                (0, 0): [  # + 2 V groups per kp slot, emitted inline below
                    lambda: emit_qk_proj(qT, wq_sb, qpT, 0, 1),
                ],
                (0, 1): [
                    lambda: emit_qk_proj(qT, wq_sb, qpT, 0, 2),
                    lambda: emit_qk_proj(qT, wq_sb, qpT, 0, 3),
                    lambda: emit_qk_proj(kT, wk_sb, kpT, 1, 0),
                    lambda: emit_qk_proj(kT, wk_sb, kpT, 1, 1),
                    lambda: emit_qk_proj(kT, wk_sb, kpT, 1, 2),
                    lambda: emit_qk_proj(kT, wk_sb, kpT, 1, 3),
                ],
                (0, 2): [
                    lambda: emit_qk_proj(qT, wq_sb, qpT, 1, 0),
                    lambda: emit_qk_proj(qT, wq_sb, qpT, 1, 1),
                    lambda: emit_qk_proj(qT, wq_sb, qpT, 1, 2),
                    lambda: emit_qk_proj(qT, wq_sb, qpT, 1, 3),
                ],
                (0, 3): [],
                (1, 0): [],  # WO h0/h1 inserted dynamically
                (1, 1): [],
                (1, 2): [],
                (1, 3): [],
            }

            hr = {}
            xh_done = {}

            for hp in range(2):
                hA, hB = 2 * hp, 2 * hp + 1
                for h in (hA, hB):
                    hr[h] = hrp.tile([64, 2048], F16, tag="hr", name=f"hr{h}")
                for qb in range(4):
                    fq = list(filler[(hp, qb)])
                    pv = {
                        h: ps_pv.tile([65, 512], F32, tag="pv", name=f"pv{h}")
                        for h in (hA, hB)
                    }
                    for kp in range(8):
                        sc = {
                            h: ps_sc.tile([128, 1024], F32, tag="sc", name=f"sc{h}")
                            for h in (hA, hB)
                        }
                        for half in range(2):
                            kt = 2 * kp + half
                            for i, h in enumerate((hA, hB)):
                                nc.tensor.matmul(
                                    sc[h][:, 512 * half : 512 * (half + 1)],
                                    kpT[64 * i : 64 * (i + 1), hp,
                                        128 * kt : 128 * (kt + 1)],
                                    qpT[64 * i : 64 * (i + 1), hp,
                                        512 * qb : 512 * (qb + 1)],
                                    start=True,
                                    stop=True,
                                    tile_position=(64 * i, 0),
                                )
                        e = {}
                        for h in (hA, hB):
                            e[h] = epool.tile([128, 1024], BF, tag="e",
                                              name=f"e{h}")
                            nc.scalar.activation(e[h], sc[h], Exp, scale=SCALE)
                        # filler between scores and P@V, where PE waits on ACT
                        if FILLER:
                            if hp == 0 and qb == 0 and kp < 7:
                                emit_v_group(2 * kp + 2)
                                emit_v_group(2 * kp + 3)
                            elif fq:
                                fq.pop(0)()
                        for half in range(2):
                            kt = 2 * kp + half
                            for h in (hA, hB):
                                nc.tensor.matmul(
                                    pv[h],
                                    vaug[:, kt, h, :],
                                    e[h][:, 512 * half : 512 * (half + 1)],
                                    start=(kt == 0),
                                    stop=(kt == 15),
                                )
                    # normalize + head/seq-mixed layout write
                    for h in (hA, hB):
                        rc = small.tile([1, 512], F32, tag="rc", name=f"rc{h}")
                        if RECIP_EXACT:
                            nc.vector.reciprocal(rc, pv[h][64:65, :])
                        else:
                            nc.vector.reciprocal_approx_fast(rc, pv[h][64:65, :])
                        rcb = small.tile([1, 512], BF, tag="rcb", name=f"rcb{h}")
                        nc.vector.tensor_copy(rcb, rc)
                        bct = ps_mix.tile([128, 512], F32, tag="mix",
                                          name=f"bct{h}")
                        bc = bct[0:64, :]
                        nc.tensor.matmul(bc, ones_bf, rcb, start=True, stop=True)
                        bc_sb = small.tile([64, 512], F32, tag="bcs",
                                           name=f"bcs{h}")
                        nc.vector.tensor_copy(bc_sb, bc)
                        hview = hr[h].rearrange("p (j r) -> p r j", j=16)[
                            :, 32 * qb : 32 * (qb + 1), :
                        ]
                        nc.vector.tensor_mul(hview, pv[h][0:64, :], bc_sb)
                    # after the last qb of pair 0: queue scatter + WO as filler
                    if FILLER and hp == 0 and qb == 3:
                        for i, h in enumerate((hA, hB)):
                            xh_done[h] = emit_scatter(h, hr[h])
                            filler[(1, i)].append(
                                lambda h=h: emit_wo(h, xh_done[h])
                            )

            # tail: remaining heads
            for h in (0, 1, 2, 3) if not FILLER else (2, 3):
                xh = emit_scatter(h, hr[h])
                emit_wo(h, xh)

    nc.finalize()
    return nc


def make_in_maps(Q, K, V, WQ, WK, WV, WO):
    in_maps = []
    wo_full = np.ascontiguousarray(WO.astype(np.float16))
    Qb = Q.astype(np.float16)
    Kb = K.astype(np.float16)
    Vb = V.astype(np.float16)
    for b in range(B):
        qTb = np.ascontiguousarray(Qb[b].T)
        kTb = np.ascontiguousarray(Kb[b].T)
        vTb = np.ascontiguousarray(Vb[b].T)
        for g in range(GROUPS):
            hs = slice(4 * g, 4 * g + 4)
            # [4, D, dk] -> [D, 4*dk]
            wqc = np.ascontiguousarray(
                WQ[hs].transpose(1, 0, 2).reshape(D, 256).astype(np.float16)
            )
            wkc = np.ascontiguousarray(
                WK[hs].transpose(1, 0, 2).reshape(D, 256).astype(np.float16)
            )
            wvc = np.ascontiguousarray(
                WV[hs].transpose(1, 0, 2).reshape(D, 256).astype(np.float16)
            )
            in_maps.append(
                {"qT": qTb, "kT": kTb, "vT": vTb,
                 "wq": wqc, "wk": wkc, "wv": wvc, "wo": wo_full}
            )
    return in_maps


def run(inputs, **run_kwargs):
    global _cached_nc
    if _cached_nc is None:
        _cached_nc = build_nc()
    in_maps = make_in_maps(**inputs)
    res = run_bass_kernel_spmd(
        _cached_nc, in_maps, core_ids=list(range(8)), **run_kwargs
    )
    full = np.zeros((B, S, D), np.float32)
    for b in range(B):
        for g in range(GROUPS):
            full[b, 512 * g : 512 * (g + 1), :] = res.results[4 * b + g]["out"]
    return full, res


def kernel(**inputs):
    full, _ = run(inputs)
    return full


if __name__ == "__main__":
    rng = np.random.default_rng(0)
    inputs = {
        "Q": rng.standard_normal((B, S, D)).astype(np.float32),
        "K": rng.standard_normal((B, S, D)).astype(np.float32),
        "V": rng.standard_normal((B, S, D)).astype(np.float32),
        "WQ": (rng.uniform(-0.1, 0.1, (H, D, DK))).astype(np.float32),
        "WK": (rng.uniform(-0.1, 0.1, (H, D, DK))).astype(np.float32),
        "WV": (rng.uniform(-0.1, 0.1, (H, D, DK))).astype(np.float32),
        "WO": (rng.uniform(-0.1, 0.1, (H * DK, D))).astype(np.float32),
    }
    out = kernel(**inputs)
    print("kernel out", out.shape, out.dtype, float(np.abs(out).max()))


# revision 15
# speedup vs baseline: 1.9114x; 1.2411x over previous
"""Trainium2 Bass kernel for nn_MultiHeadAttention (B=2, S=2048, D=1024, H=16, dk=64).

Sharding: 8 cores = (batch b in {0,1}) x (head group g in {0..3}, 4 heads each).
The reference's RAW reshape (B,H,S,dk) -> (B,S,H*dk) means output row
s' = h*128 + s//16 depends only on head h, so core (b,g) produces output rows
[512g, 512(g+1)) of batch b -- pure concatenation, no collectives.

v1 (bf16): all matmul operands bf16 (fp32 runs the PE in 4-pass HIGH mode;
bf16 streams 1 column/cycle), softmax denominators via the augmented-ones
column of V, reciprocal via the fast-approx DVE op, and the whole kernel is
software-pipelined: projections and output matmuls are interleaved as filler
into the ACT-paced attention loop so the tensor engine never idles.

Per-core pipeline:
  1. QpT/KpT = W^T @ x^T  -> [128(=2 heads*dk), 2(head pair), S] bf16
     Vaug    = x^T.T @ Wv -> [128 kpos-chunk, 16 kt, 4 heads, 64+1] bf16
     (ones column -> softmax denominator accumulates in the P@V matmul)
  2. Per head pair, per 512-wide q block: scores^T tiles on PE (2 heads
     packed via row tile_position), exp via ACT (scale fused, no max
     subtraction -- fp32 PSUM scores, bf16 exp output), P@V with V_aug
     -> PSUM [65, 512], row 64 = denominator.
  3. recip_approx_fast (DVE) -> bf16 -> broadcast via K=1 outer-product
     matmul -> normalize on DVE, writing head/seq-mixed layout directly.
  4. One scatter DMA per (head, partition parity) assembles X^T tiles;
     WO matmul per head; DMA out fp32.
"""

import sys

try:
    import concourse.bass as bass  # noqa: F401
except ImportError:
    sys.path.insert(0, "/opt/trn_rl_repo")

import os

import numpy as np
from ml_dtypes import bfloat16

FILLER = os.environ.get("K_FILLER", "1") == "1"
# "approx": copy denom row to SBUF then reciprocal_approx_fast (fast path;
# approx direct from PSUM returned garbage on HW). "exact": nc.vector.reciprocal.
RECIP_MODE = os.environ.get("K_RECIP", "approx")

import concourse.bacc as bacc
import concourse.tile as tile
from concourse import mybir
from concourse.bass_utils import run_bass_kernel_spmd

BF = mybir.dt.bfloat16
F16 = mybir.dt.float16
F32 = mybir.dt.float32

B, S, D, H, DK = 2, 2048, 1024, 16, 64
GROUPS = 4
SCALE = 1.0 / 8.0  # 1/sqrt(dk)
E_BUFS = 12

_cached_nc = None


def build_nc():
    nc = bacc.Bacc(None, target_bir_lowering=False)
    qT = nc.dram_tensor("qT", [D, S], F16, kind="ExternalInput")
    kT = nc.dram_tensor("kT", [D, S], F16, kind="ExternalInput")
    vT = nc.dram_tensor("vT", [D, S], F16, kind="ExternalInput")
    wq = nc.dram_tensor("wq", [D, 256], F16, kind="ExternalInput")
    wk = nc.dram_tensor("wk", [D, 256], F16, kind="ExternalInput")
    wv = nc.dram_tensor("wv", [D, 256], F16, kind="ExternalInput")
    wo = nc.dram_tensor("wo", [D, D], F16, kind="ExternalInput")
    out = nc.dram_tensor("out", [512, D], F32, kind="ExternalOutput")

    Exp = mybir.ActivationFunctionType.Exp

    with tile.TileContext(nc) as tc, nc.allow_low_precision(
        reason="bf16 matmuls with fp32 PSUM accumulation; attention weight "
        "and normalization rounding averages out over 2048 positions"
    ):
        with (
            tc.tile_pool(name="persist", bufs=1) as persist,
            tc.tile_pool(name="wqkv", bufs=1) as wqkv,
            tc.tile_pool(name="hrp", bufs=4) as hrp,
            tc.tile_pool(name="xhp", bufs=2) as xhp,
            tc.tile_pool(name="small", bufs=4) as small,
            tc.tile_pool(name="opool", bufs=2) as opool,
            tc.tile_pool(name="epool", bufs=E_BUFS) as epool,
            tc.tile_pool(name="stream", bufs=3) as stream,
            tc.tile_pool(name="vstream", bufs=2) as vstream,
            tc.tile_pool(name="ps_mix", bufs=1, space="PSUM") as ps_mix,
            tc.tile_pool(name="ps_sc", bufs=2, space="PSUM") as ps_sc,
            tc.tile_pool(name="ps_pv", bufs=3, space="PSUM") as ps_pv,
        ):
            qpT = persist.tile([128, 2, S], F16, tag="qpT")
            kpT = persist.tile([128, 2, S], F16, tag="kpT")
            vaug = persist.tile([128, 16, 4, 65], BF, tag="vaug")
            ones_f32 = persist.tile([128, 1], F32, tag="ones_f32")
            nc.vector.memset(ones_f32, 1.0)
            nc.vector.tensor_copy(
                vaug[:, :, :, 64:65], ones_f32.to_broadcast((128, 16, 4, 1))
            )
            ones_bf = persist.tile([1, 64], BF, tag="ones_bf")
            nc.vector.tensor_copy(ones_bf, ones_f32[0:1, :].to_broadcast((1, 64)))

            wq_sb = wqkv.tile([128, 8, 256], F16, tag="wq")
            wk_sb = wqkv.tile([128, 8, 256], F16, tag="wk")
            wv_sb = wqkv.tile([128, 8, 256], F16, tag="wv")
            wo_sb = wqkv.tile([128, 8, D], F16, tag="wo")
            for w_dram, w_sb in ((wk, wk_sb), (wq, wq_sb), (wv, wv_sb)):
                nc.sync.dma_start(
                    out=w_sb, in_=w_dram.rearrange("(t p) n -> p t n", p=128)
                )

            # --- emission helpers; each is one "filler group" (~1us PE) ---

            def emit_qk_proj(x_dram, w_sb, outt, m, nb):
                """One [128, 512] block of a Q/K projection for head pair m."""
                st = stream.tile([128, 8, 512], F16, tag="acts", name="stq")
                nc.sync.dma_start(
                    out=st,
                    in_=x_dram.rearrange("(t p) s -> p t s", p=128)[
                        :, :, 512 * nb : 512 * (nb + 1)
                    ],
                )
                ps = ps_mix.tile([128, 512], F32, tag="mix", name="psq")
                for k in range(8):
                    nc.tensor.matmul(
                        ps,
                        w_sb[:, k, 128 * m : 128 * (m + 1)],
                        st[:, k, :],
                        start=(k == 0),
                        stop=(k == 7),
                    )
                nc.vector.tensor_copy(outt[:, m, 512 * nb : 512 * (nb + 1)], ps)

            v_st = {}

            def emit_v_group(kt):
                """V projection for one 128-row kpos chunk kt (all 4 heads)."""
                nb = kt // 4
                if nb not in v_st:
                    st = vstream.tile([128, 8, 512], F16, tag="vacts", name="stv")
                    nc.sync.dma_start(
                        out=st,
                        in_=vT.rearrange("(t p) s -> p t s", p=128)[
                            :, :, 512 * nb : 512 * (nb + 1)
                        ],
                    )
                    v_st[nb] = st
                st = v_st[nb]
                sti = kt % 4
                ps_full = ps_mix.tile([128, 512], F32, tag="mix", name="vps")
                ps = ps_full[:, :256]
                for k in range(8):
                    nc.tensor.matmul(
                        ps,
                        st[:, k, 128 * sti : 128 * (sti + 1)],
                        wv_sb[:, k, :],
                        start=(k == 0),
                        stop=(k == 7),
                    )
                nc.vector.tensor_copy(
                    vaug[:, kt, :, 0:64], ps.rearrange("p (h c) -> p h c", h=4)
                )

            def emit_wo(h, xh):
                """Output projection for head h from assembled X^T tiles."""
                for n in range(2):
                    wops = ps_mix.tile([128, 512], F32, tag="mix", name=f"wops{h}")
                    for t in range(8):
                        nc.tensor.matmul(
                            wops,
                            xh[:, t, :],
                            wo_sb[:, t, 512 * n : 512 * (n + 1)],
                            start=(t == 0),
                            stop=(t == 7),
                        )
                    ot = opool.tile([128, 512], F32, tag="o", name=f"ot{h}")
                    nc.vector.tensor_copy(ot, wops)
                    nc.sync.dma_start(
                        out=out[128 * h : 128 * (h + 1), 512 * n : 512 * (n + 1)],
                        in_=ot,
                    )

            def emit_scatter(h, hr_h):
                """hr (head-transposed) -> X^T chunks for the WO lhsT."""
                xh = xhp.tile([128, 8, 128], F16, tag="xh", name=f"xh{h}")
                hv = hr_h.rearrange("p (j r) -> p j r", j=16)
                for par in range(2):
                    nc.sync.dma_start(
                        out=xh[64 * par : 64 * (par + 1)], in_=hv[:, par::2, :]
                    )
                return xh

            # ---------------- software-pipelined main schedule ----------------
            # Pre-phase: K proj (pair 0), Q proj (pair 0, block 0), V kt 0-1.
            for nb in range(4):
                emit_qk_proj(kT, wk_sb, kpT, 0, nb)
            emit_qk_proj(qT, wq_sb, qpT, 0, 0)
            emit_v_group(0)
            emit_v_group(1)
            if not FILLER:
                for kt in range(2, 16):
                    emit_v_group(kt)
                for nb in range(1, 4):
                    emit_qk_proj(qT, wq_sb, qpT, 0, nb)
                for nb in range(4):
                    emit_qk_proj(kT, wk_sb, kpT, 1, nb)
                for nb in range(4):
                    emit_qk_proj(qT, wq_sb, qpT, 1, nb)
            # wo needed ~150us in; emitted here so it doesn't delay kT/qT
            nc.sync.dma_start(
                out=wo_sb, in_=wo.rearrange("(t p) n -> p t n", p=128)
            )

            # Filler queues per (hp, qb) attention block: list of closures.
            # RAW constraint: a filler producing qpT/kpT block X must be
            # emitted in a block STRICTLY BEFORE the attention block whose
            # scores read X (engines run in-order; qb's sc matmuls are
            # emitted before qb's fillers).
            filler = {
                (0, 0): [  # + 2 V groups per kp slot, emitted inline below
                    lambda: emit_qk_proj(qT, wq_sb, qpT, 0, 1),
                ],
                (0, 1): [
                    lambda: emit_qk_proj(qT, wq_sb, qpT, 0, 2),
                    lambda: emit_qk_proj(qT, wq_sb, qpT, 0, 3),
                    lambda: emit_qk_proj(kT, wk_sb, kpT, 1, 0),
                    lambda: emit_qk_proj(kT, wk_sb, kpT, 1, 1),
                    lambda: emit_qk_proj(kT, wk_sb, kpT, 1, 2),
                    lambda: emit_qk_proj(kT, wk_sb, kpT, 1, 3),
                ],
                (0, 2): [
                    lambda: emit_qk_proj(qT, wq_sb, qpT, 1, 0),
                    lambda: emit_qk_proj(qT, wq_sb, qpT, 1, 1),
                    lambda: emit_qk_proj(qT, wq_sb, qpT, 1, 2),
                    lambda: emit_qk_proj(qT, wq_sb, qpT, 1, 3),
                ],
                (0, 3): [],
                (1, 0): [],  # WO h0/h1 inserted dynamically
                (1, 1): [],
                (1, 2): [],
                (1, 3): [],
            }

            hr = {}
            xh_done = {}

            for hp in range(2):
                hA, hB = 2 * hp, 2 * hp + 1
                for h in (hA, hB):
                    hr[h] = hrp.tile([64, 2048], F16, tag="hr", name=f"hr{h}")
                for qb in range(4):
                    fq = list(filler[(hp, qb)])
                    pv = {
                        h: ps_pv.tile([65, 512], F32, tag="pv", name=f"pv{h}")
                        for h in (hA, hB)
                    }
                    e_q = {}

                    def emit_sc(kp, hp=hp, qb=qb, hA=hA, hB=hB, e_q=None):
                        sc = {
                            h: ps_sc.tile([128, 1024], F32, tag="sc",
                                          name=f"sc{h}")
                            for h in (hA, hB)
                        }
                        for half in range(2):
                            kt = 2 * kp + half
                            for i, h in enumerate((hA, hB)):
                                nc.tensor.matmul(
                                    sc[h][:, 512 * half : 512 * (half + 1)],
                                    kpT[64 * i : 64 * (i + 1), hp,
                                        128 * kt : 128 * (kt + 1)],
                                    qpT[64 * i : 64 * (i + 1), hp,
                                        512 * qb : 512 * (qb + 1)],
                                    start=True,
                                    stop=True,
                                    tile_position=(64 * i, 0),
                                )
                        for h in (hA, hB):
                            et = epool.tile([128, 1024], BF, tag="e",
                                            name=f"e{h}")
                            nc.scalar.activation(et, sc[h], Exp, scale=SCALE)
                            e_q[(kp, h)] = et

                    def emit_pv(kp, pv=pv, hA=hA, hB=hB, e_q=None):
                        for half in range(2):
                            kt = 2 * kp + half
                            for h in (hA, hB):
                                nc.tensor.matmul(
                                    pv[h],
                                    vaug[:, kt, h, :],
                                    e_q[(kp, h)][:,
                                                 512 * half : 512 * (half + 1)],
                                    start=(kt == 0),
                                    stop=(kt == 15),
                                )

                    # sc runs SKEW blocks ahead of pv so the PE never parks
                    # on the pv accumulation right at a qb boundary (the
                    # normalize chain of the previous qb is still draining).
                    SKEW = 2
                    for kp in range(8):
                        emit_sc(kp, e_q=e_q)
                        # filler between scores and P@V, where PE waits on ACT
                        if FILLER:
                            if hp == 0 and qb == 0 and kp < 7:
                                emit_v_group(2 * kp + 2)
                                emit_v_group(2 * kp + 3)
                            elif fq:
                                fq.pop(0)()
                        if kp >= SKEW:
                            emit_pv(kp - SKEW, e_q=e_q)
                    for kp in range(8 - SKEW, 8):
                        emit_pv(kp, e_q=e_q)
                    # normalize + head/seq-mixed layout write.  q columns were
                    # permuted host-side to j-major within each 512 block, so
                    # both the pv read and the hr write are contiguous runs.
                    for h in (hA, hB):
                        rc = small.tile([1, 512], F32, tag="rc", name=f"rc{h}")
                        if RECIP_MODE == "exact":
                            nc.vector.reciprocal(rc, pv[h][64:65, :])
                        else:
                            dn = small.tile([1, 512], F32, tag="dn",
                                            name=f"dn{h}")
                            nc.vector.tensor_copy(dn, pv[h][64:65, :])
                            nc.vector.reciprocal_approx_fast(rc, dn)
                        rcb = small.tile([1, 512], BF, tag="rcb", name=f"rcb{h}")
                        nc.vector.tensor_copy(rcb, rc)
                        bct = ps_mix.tile([128, 512], F32, tag="mix",
                                          name=f"bct{h}")
                        bc = bct[0:64, :]
                        nc.tensor.matmul(bc, ones_bf, rcb, start=True, stop=True)
                        bc_sb = small.tile([64, 512], F32, tag="bcs",
                                           name=f"bcs{h}")
                        nc.vector.tensor_copy(bc_sb, bc)
                        hview = hr[h].rearrange("p (j r) -> p j r", j=16)[
                            :, :, 32 * qb : 32 * (qb + 1)
                        ]
                        nc.vector.tensor_mul(hview, pv[h][0:64, :], bc_sb)
                    # after the last qb of pair 0: queue scatter + WO as filler
                    if FILLER and hp == 0 and qb == 3:
                        for i, h in enumerate((hA, hB)):
                            xh_done[h] = emit_scatter(h, hr[h])
                            filler[(1, i)].append(
                                lambda h=h: emit_wo(h, xh_done[h])
                            )

            # tail: remaining heads
            for h in (0, 1, 2, 3) if not FILLER else (2, 3):
                xh = emit_scatter(h, hr[h])
                emit_wo(h, xh)

    nc.finalize()
    return nc


_QPERM = None


def _qperm():
    """Permute q columns j-major within each 512 block: position j*32+r holds
    original offset r*16+j.  Makes the normalize write into hr's (j r) layout
    contiguous; everything downstream of the scores rhs follows the permuted
    order consistently, and the output mapping is unchanged."""
    global _QPERM
    if _QPERM is None:
        p = np.arange(512)
        perm = (p % 32) * 16 + p // 32
        _QPERM = np.concatenate([512 * qb + perm for qb in range(4)])
    return _QPERM


def make_in_maps(Q, K, V, WQ, WK, WV, WO):
    in_maps = []
    wo_full = np.ascontiguousarray(WO.astype(np.float16))
    Qb = Q[:, _qperm(), :].astype(np.float16)
    Kb = K.astype(np.float16)
    Vb = V.astype(np.float16)
    for b in range(B):
        qTb = np.ascontiguousarray(Qb[b].T)
        kTb = np.ascontiguousarray(Kb[b].T)
        vTb = np.ascontiguousarray(Vb[b].T)
        for g in range(GROUPS):
            hs = slice(4 * g, 4 * g + 4)
            # [4, D, dk] -> [D, 4*dk]
            wqc = np.ascontiguousarray(
                WQ[hs].transpose(1, 0, 2).reshape(D, 256).astype(np.float16)
            )
            wkc = np.ascontiguousarray(
                WK[hs].transpose(1, 0, 2).reshape(D, 256).astype(np.float16)
            )
            wvc = np.ascontiguousarray(
                WV[hs].transpose(1, 0, 2).reshape(D, 256).astype(np.float16)
            )
            in_maps.append(
                {"qT": qTb, "kT": kTb, "vT": vTb,
                 "wq": wqc, "wk": wkc, "wv": wvc, "wo": wo_full}
            )
    return in_maps


def run(inputs, **run_kwargs):
    global _cached_nc
    if _cached_nc is None:
        _cached_nc = build_nc()
    in_maps = make_in_maps(**inputs)
    res = run_bass_kernel_spmd(
        _cached_nc, in_maps, core_ids=list(range(8)), **run_kwargs
    )
    full = np.zeros((B, S, D), np.float32)
    for b in range(B):
        for g in range(GROUPS):
            full[b, 512 * g : 512 * (g + 1), :] = res.results[4 * b + g]["out"]
    return full, res


def kernel(**inputs):
    full, _ = run(inputs)
    return full


if __name__ == "__main__":
    rng = np.random.default_rng(0)
    inputs = {
        "Q": rng.standard_normal((B, S, D)).astype(np.float32),
        "K": rng.standard_normal((B, S, D)).astype(np.float32),
        "V": rng.standard_normal((B, S, D)).astype(np.float32),
        "WQ": (rng.uniform(-0.1, 0.1, (H, D, DK))).astype(np.float32),
        "WK": (rng.uniform(-0.1, 0.1, (H, D, DK))).astype(np.float32),
        "WV": (rng.uniform(-0.1, 0.1, (H, D, DK))).astype(np.float32),
        "WO": (rng.uniform(-0.1, 0.1, (H * DK, D))).astype(np.float32),
    }
    out = kernel(**inputs)
    print("kernel out", out.shape, out.dtype, float(np.abs(out).max()))
